# revision 1
# baseline (speedup 1.0000x reference)
"""HQQ 4-bit quantized linear layer on 8 Trainium2 NeuronCores.

Reference computation:
    W_r = concat([W_q >> 4, W_q & 0xF], axis=0).astype(f32)    # [64, 704512]
    W   = ((W_r - zero) * scale).reshape(11008, 4096)          # [out, in]
    out = x @ W.T + bias                                        # [4, 2048, 11008]

Group structure: group j = r*4096 + k (r in [0,172), k in [0,4096)) supplies
output feature o = i*172 + r (element i in [0,64) of the group) at input
feature k.  So for a fixed o, every k belongs to a different group, and
W[o, k] = (nib[i, j] - zero[j]) * scale[j] with i = o//172, j = (o%172)*4096+k.

Sharding (column-parallel over output features, SPMD-uniform):
  core c owns W_q byte-rows [4c, 4c+4).  Byte-row b holds the high nibble of
  i=b and the low nibble of i=b+32, so core c produces output features
  o in {(4c+ib)*172 + r} (high) and {(32+4c+ib)*172 + r} (low), ib in [0,4),
  r in [0,172): 1376 contiguous-after-gather features per core.  Every core
  runs the identical program (extract high AND low nibbles of its 4 rows);
  x / scale / zero are replicated.

Device kernel (per core):
  - dequantize the 4096x1376 weight shard once into SBUF as bf16, with
    k (input feature) on partitions, via
      w = (wq >> 4) * scale_bcast - (zero*scale)_bcast     (and & 0xF)
  - stream x^T tiles [k=128, tokens], cast f32->bf16 on ScalarE,
    matmul-accumulate over 32 k-tiles into PSUM (tokens on psum partitions),
  - drain PSUM + bias (broadcast tile) on VectorE, DMA out f32.
"""

import os
import sys

for _p in ("/opt/trn_rl_repo",):
    if os.path.isdir(_p) and _p not in sys.path:
        sys.path.insert(0, _p)

import numpy as np

P = 128
IN_F = 4096
OUT_F = 11008
GROUP = 64
R_FULL = 172          # OUT_F // GROUP
IB_FULL = 4           # W_q byte rows per core
N_CORES = 8
NTOK_FULL = 8192      # 4 * 2048


def _chunks(n, maxc=512):
    out = []
    off = 0
    while off < n:
        sz = min(maxc, n - off)
        out.append((off, sz))
        off += sz
    return out


def build_program(KT=32, NSUP=32, SUP=256, IB=IB_FULL, R=R_FULL, num_devices=N_CORES):
    """Build the SPMD bass program. Returns the compiled Bacc object.

    KT: number of 128-wide k tiles (K = 128*KT)
    NSUP: number of token supersteps;  SUP: tokens per superstep (mult of 128)
    IB: W_q byte rows per core;  R: group minor dim (o = i*R + r)
    """
    import concourse.bacc as bacc
    import concourse.bass as bass
    import concourse.mybir as mybir
    import concourse.tile as tile
    from concourse.alu_op_type import AluOpType

    f32 = mybir.dt.float32
    bf16 = mybir.dt.bfloat16
    u8 = mybir.dt.uint8

    K = KT * P
    NTOK = NSUP * SUP
    NSUB = SUP // P
    OHALF = IB * R
    OFULL = 2 * OHALF
    CHUNKS = _chunks(OFULL)

    nc = bacc.Bacc(
        "TRN2", target_bir_lowering=False, debug=False, num_devices=num_devices
    )

    xt = nc.dram_tensor("xt", [K, NTOK], f32, kind="ExternalInput")
    wq = nc.dram_tensor("wq", [K, OHALF], u8, kind="ExternalInput")
    scale_t = nc.dram_tensor("scale_t", [K, R], f32, kind="ExternalInput")
    zero_t = nc.dram_tensor("zero_t", [K, R], f32, kind="ExternalInput")
    bias = nc.dram_tensor("bias", [OFULL], f32, kind="ExternalInput")
    out = nc.dram_tensor("out", [NTOK, OFULL], f32, kind="ExternalOutput")

    with tile.TileContext(nc) as tc:
        with (
            tc.tile_pool(name="cst", bufs=1) as cst,
            tc.tile_pool(name="wres", bufs=1) as wres,
            tc.tile_pool(name="dq", bufs=3) as dq,
            tc.tile_pool(name="xload", bufs=6) as xp,
            tc.tile_pool(name="xb", bufs=2) as xbp,
            tc.tile_pool(name="psum", bufs=2, space="PSUM") as pp,
            tc.tile_pool(name="outp", bufs=3) as op,
        ):
            # bias broadcast to [128, OFULL] via partition-step-0 DMA read
            bias_b = cst.tile([P, OFULL], f32)
            bias_bcast_src = bass.AP(bias, 0, [[0, P], [1, OFULL]])
            nc.sync.dma_start(out=bias_b[:], in_=bias_bcast_src)

            # --- dequantize the whole weight shard into resident SBUF bf16 ---
            w_res = [
                wres.tile([P, OFULL], bf16, tag=f"w{kt}", name=f"w{kt}")
                for kt in range(KT)
            ]
            for kt in range(KT):
                ks = slice(kt * P, (kt + 1) * P)
                wq_t = dq.tile([P, OHALF], u8, tag="wq")
                nc.sync.dma_start(out=wq_t[:], in_=wq[ks, :])
                sc = dq.tile([P, R], f32, tag="sc")
                nc.sync.dma_start(out=sc[:], in_=scale_t[ks, :])
                zr = dq.tile([P, R], f32, tag="zr")
                nc.sync.dma_start(out=zr[:], in_=zero_t[ks, :])
                # broadcast [P, R] -> [P, IB, R] with stride-0 middle dim
                sc_b = bass.AP(sc.tensor, sc.offset, [sc.ap[0], [0, IB], [1, R]])
                zr_b = bass.AP(zr.tensor, zr.offset, [zr.ap[0], [0, IB], [1, R]])

                for half, (op0, scl) in enumerate(
                    [(AluOpType.logical_shift_right, 4), (AluOpType.bitwise_and, 15)]
                ):
                    nibu = dq.tile([P, OHALF], u8, tag=f"nibu{half}", name=f"nibu{half}")
                    nc.vector.tensor_scalar(
                        out=nibu[:], in0=wq_t[:], scalar1=scl, scalar2=None, op0=op0
                    )
                    nib = dq.tile([P, OHALF], f32, tag=f"nib{half}", name=f"nib{half}")
                    nc.vector.tensor_tensor(
                        out=nib[:], in0=nibu[:], in1=zr_b, op=AluOpType.subtract
                    )
                    nc.vector.tensor_tensor(
                        out=w_res[kt][:, half * OHALF:(half + 1) * OHALF],
                        in0=nib[:], in1=sc_b, op=AluOpType.mult,
                    )

            # --- main GEMM loop ---
            for ns in range(NSUP):
                tok0 = ns * SUP
                xb = xbp.tile([P, KT, SUP], bf16, tag="xb")
                for kt in range(KT):
                    xf = xp.tile([P, SUP], f32, tag="xf")
                    nc.sync.dma_start(
                        out=xf[:], in_=xt[kt * P:(kt + 1) * P, tok0:tok0 + SUP]
                    )
                    # f32 -> bf16 cast on ScalarE (ACT otherwise idle)
                    nc.scalar.copy(out=xb[:, kt, :], in_=xf[:])
                for sub in range(NSUB):
                    ps = [
                        pp.tile([P, sz], f32, tag=f"ps{ci}", name=f"ps{ci}")
                        for ci, (off, sz) in enumerate(CHUNKS)
                    ]
                    for kt in range(KT):
                        lhsT = xb[:, kt, sub * P:(sub + 1) * P]
                        for ci, (off, sz) in enumerate(CHUNKS):
                            nc.tensor.matmul(
                                ps[ci][:],
                                lhsT,
                                w_res[kt][:, off:off + sz],
                                start=(kt == 0),
                                stop=(kt == KT - 1),
                            )
                    ot = op.tile([P, OFULL], f32, tag="ot")
                    for ci, (off, sz) in enumerate(CHUNKS):
                        nc.vector.tensor_add(
                            out=ot[:, off:off + sz], in0=ps[ci][:],
                            in1=bias_b[:, off:off + sz],
                        )
                    row0 = tok0 + sub * P
                    nc.sync.dma_start(out=out[row0:row0 + P, :], in_=ot[:])

    nc.compile()
    return nc


_PROG_CACHE = {}


def _get_program():
    key = "full"
    if key not in _PROG_CACHE:
        _PROG_CACHE[key] = build_program()
    return _PROG_CACHE[key]


def shard_inputs(x, W_q, scale, zero, bias):
    """Host-side sharding / layout transforms (no arithmetic on values)."""
    x = np.asarray(x, dtype=np.float32)
    W_q = np.asarray(W_q)
    scale = np.asarray(scale, dtype=np.float32)
    zero = np.asarray(zero, dtype=np.float32)
    bias = np.asarray(bias, dtype=np.float32)

    ntok = x.shape[0] * x.shape[1]
    xt = np.ascontiguousarray(x.reshape(ntok, IN_F).T)              # [K, NTOK]
    scale_t = np.ascontiguousarray(scale.reshape(R_FULL, IN_F).T)   # [K, R]
    zero_t = np.ascontiguousarray(zero.reshape(R_FULL, IN_F).T)     # [K, R]
    wq_u8 = W_q.astype(np.uint8)                                    # values < 256
    bias_rs = bias.reshape(GROUP, R_FULL)                           # [i, r]

    in_maps = []
    for c in range(N_CORES):
        rows = wq_u8[IB_FULL * c: IB_FULL * (c + 1)]                # [4, 704512]
        # [ib, r, k] -> [k, ib, r] -> [K, OHALF]
        wq_c = np.ascontiguousarray(
            rows.reshape(IB_FULL, R_FULL, IN_F).transpose(2, 0, 1)
        ).reshape(IN_F, IB_FULL * R_FULL)
        bias_c = np.concatenate(
            [
                bias_rs[IB_FULL * c: IB_FULL * (c + 1)].ravel(),
                bias_rs[32 + IB_FULL * c: 32 + IB_FULL * (c + 1)].ravel(),
            ]
        )
        in_maps.append(
            {
                "xt": xt,
                "wq": wq_c,
                "scale_t": scale_t,
                "zero_t": zero_t,
                "bias": bias_c,
            }
        )
    return in_maps


def gather_output(results, ntok=NTOK_FULL):
    out = np.empty((ntok, OUT_F), dtype=np.float32)
    ohalf = IB_FULL * R_FULL
    for c in range(N_CORES):
        res = results[c]["out"]
        lo = IB_FULL * c * R_FULL
        out[:, lo: lo + ohalf] = res[:, :ohalf]
        lo = (32 + IB_FULL * c) * R_FULL
        out[:, lo: lo + ohalf] = res[:, ohalf:]
    return out


def kernel(x, W_q, scale, zero, bias):
    from concourse.bass_utils import run_bass_kernel_spmd

    x = np.asarray(x)
    b, s, _ = x.shape
    nc = _get_program()
    in_maps = shard_inputs(x, W_q, scale, zero, bias)
    res = run_bass_kernel_spmd(nc, in_maps, list(range(N_CORES)))
    out = gather_output(res.results)
    return out.reshape(b, s, OUT_F)



# revision 2
# speedup vs baseline: 1.0239x; 1.0239x over previous
"""HQQ 4-bit quantized linear layer on 8 Trainium2 NeuronCores.

Reference computation:
    W_r = concat([W_q >> 4, W_q & 0xF], axis=0).astype(f32)    # [64, 704512]
    W   = ((W_r - zero) * scale).reshape(11008, 4096)          # [out, in]
    out = x @ W.T + bias                                        # [4, 2048, 11008]

Group structure: group j = r*4096 + k (r in [0,172), k in [0,4096)) supplies
output feature o = i*172 + r (element i in [0,64) of the group) at input
feature k.  So for a fixed o, every k belongs to a different group, and
W[o, k] = (nib[i, j] - zero[j]) * scale[j] with i = o//172, j = (o%172)*4096+k.
Both nibbles of a byte belong to the same group j (rows i_b and i_b+32), so
they share zero/scale.

Sharding (column-parallel over output features, SPMD-uniform):
  core c owns W_q byte-rows [4c, 4c+4), producing output features
  o in {(4c+ib)*172 + r} (high nibble) and {(32+4c+ib)*172 + r} (low),
  ib in [0,4), r in [0,172): 1376 features per core. x/scale/zero replicated.

Device kernel (per core):
  - banded dequant (KTB k-tiles per band) into resident SBUF bf16 weights:
      DVE:  nibble extract (u8 TS, 1x)
      ACT:  u8 -> bf16 casts, f32 -> bf16 casts of scale/zero
      DVE:  bf16 (nib - zero) and (* scale), 2x-mode eligible, in-place in w
    software-pipelined one band ahead so DVE never waits on ACT.
  - stream x^T tiles [k=128, tokens], cast f32->bf16 (GPSIMD tensor_copy),
    matmul-accumulate over 32 k-tiles into PSUM (tokens on psum partitions),
  - drain PSUM + bias on DVE per 512-wide chunk, DMA out f32 per chunk.
"""

import os
import sys

for _p in ("/opt/trn_rl_repo",):
    if os.path.isdir(_p) and _p not in sys.path:
        sys.path.insert(0, _p)

import numpy as np

P = 128
IN_F = 4096
OUT_F = 11008
GROUP = 64
R_FULL = 172          # OUT_F // GROUP
IB_FULL = 4           # W_q byte rows per core
N_CORES = 8
NTOK_FULL = 8192      # 4 * 2048

USE_GPS_XCAST = True  # f32->bf16 x casts on GPSIMD (else ACT)


def _chunks(n, maxc=512):
    out = []
    off = 0
    while off < n:
        sz = min(maxc, n - off)
        out.append((off, sz))
        off += sz
    return out


def build_program(KT=32, NSUP=32, SUP=256, IB=IB_FULL, R=R_FULL,
                  num_devices=N_CORES, KTB=4):
    """Build the SPMD bass program. Returns the compiled Bacc object."""
    import concourse.bacc as bacc
    import concourse.bass as bass
    import concourse.mybir as mybir
    import concourse.tile as tile
    from concourse.alu_op_type import AluOpType

    f32 = mybir.dt.float32
    bf16 = mybir.dt.bfloat16
    u8 = mybir.dt.uint8

    K = KT * P
    NTOK = NSUP * SUP
    NSUB = SUP // P
    OHALF = IB * R
    OFULL = 2 * OHALF
    NBAND = KT // KTB
    CHUNKS = _chunks(OFULL)

    nc = bacc.Bacc(
        "TRN2", target_bir_lowering=False, debug=False, num_devices=num_devices
    )

    xt = nc.dram_tensor("xt", [K, NTOK], f32, kind="ExternalInput")
    wq = nc.dram_tensor("wq", [K, OHALF], u8, kind="ExternalInput")
    scale_t = nc.dram_tensor("scale_t", [K, R], f32, kind="ExternalInput")
    zero_t = nc.dram_tensor("zero_t", [K, R], f32, kind="ExternalInput")
    bias = nc.dram_tensor("bias", [OFULL], f32, kind="ExternalInput")
    out = nc.dram_tensor("out", [NTOK, OFULL], f32, kind="ExternalOutput")

    with tile.TileContext(nc) as tc:
        with (
            tc.tile_pool(name="cst", bufs=1) as cst,
            tc.tile_pool(name="wres", bufs=1) as wres,
            tc.tile_pool(name="dq", bufs=2) as dq,
            tc.tile_pool(name="xload", bufs=6) as xp,
            tc.tile_pool(name="xb", bufs=2) as xbp,
            tc.tile_pool(name="psum", bufs=2, space="PSUM") as pp,
            tc.tile_pool(name="outp", bufs=3) as op,
        ):
            w_bands = [
                wres.tile([P, KTB, OFULL], bf16, tag=f"w{b}", name=f"w{b}")
                for b in range(NBAND)
            ]

            # --- banded dequant: DMA + TS (extract) + ACT cast + 2x bf16 TTs
            band_state = []

            def band_dma(b):
                k0 = b * KTB * P
                wq_t = dq.tile([P, KTB, OHALF], u8, tag="wq")
                nc.sync.dma_start(
                    out=wq_t[:],
                    in_=bass.AP(wq, k0 * OHALF,
                                [[OHALF, P], [P * OHALF, KTB], [1, OHALF]]),
                )
                sc = dq.tile([P, KTB, R], f32, tag="sc")
                nc.sync.dma_start(
                    out=sc[:],
                    in_=bass.AP(scale_t, k0 * R, [[R, P], [P * R, KTB], [1, R]]),
                )
                zr = dq.tile([P, KTB, R], f32, tag="zr")
                nc.sync.dma_start(
                    out=zr[:],
                    in_=bass.AP(zero_t, k0 * R, [[R, P], [P * R, KTB], [1, R]]),
                )
                return wq_t, sc, zr

            def band_extract(b, wq_t, sc, zr):
                # DVE: nibble extract u8->u8 (1x mode, unavoidable)
                nibu_hi = dq.tile([P, KTB, OHALF], u8, tag="nh", name=f"nh{b}")
                nc.vector.tensor_scalar(
                    out=nibu_hi[:], in0=wq_t[:], scalar1=4, scalar2=None,
                    op0=AluOpType.logical_shift_right,
                )
                nibu_lo = dq.tile([P, KTB, OHALF], u8, tag="nl", name=f"nl{b}")
                nc.vector.tensor_scalar(
                    out=nibu_lo[:], in0=wq_t[:], scalar1=15, scalar2=None,
                    op0=AluOpType.bitwise_and,
                )
                # ACT: bf16 casts of scale/zero and nibbles
                sc16 = dq.tile([P, KTB, R], bf16, tag="sc16")
                nc.scalar.copy(out=sc16[:], in_=sc[:])
                zr16 = dq.tile([P, KTB, R], bf16, tag="zr16")
                nc.scalar.copy(out=zr16[:], in_=zr[:])
                w_b = w_bands[b]
                for half, nibu in ((0, nibu_hi), (1, nibu_lo)):
                    nc.scalar.copy(
                        out=w_b[:, :, half * OHALF:(half + 1) * OHALF],
                        in_=nibu[:],
                    )
                return sc16, zr16

            def band_scalefix(b, sc16, zr16):
                # DVE: in-place (w - zero) * scale on the bf16 w tile, 2x mode
                w_b = w_bands[b]
                zr_bc = bass.AP(zr16.tensor, zr16.offset,
                                [zr16.ap[0], [R, KTB], [0, IB], [1, R]])
                sc_bc = bass.AP(sc16.tensor, sc16.offset,
                                [sc16.ap[0], [R, KTB], [0, IB], [1, R]])
                for half in (0, 1):
                    wslice = bass.AP(
                        w_b.tensor, w_b.offset + half * OHALF,
                        [w_b.ap[0], [OFULL, KTB], [R, IB], [1, R]],
                    )
                    nc.vector.tensor_tensor(
                        out=wslice, in0=wslice, in1=zr_bc, op=AluOpType.subtract
                    )
                    nc.vector.tensor_tensor(
                        out=wslice, in0=wslice, in1=sc_bc, op=AluOpType.mult
                    )

            # software pipeline: extract(b+1) is emitted before scalefix(b)
            # so DVE's TS of the next band runs while ACT casts band b.
            pend = None
            for b in range(NBAND):
                wq_t, sc, zr = band_dma(b)
                if b == 0:
                    # bias broadcast DMA right after band0's (partition-step-0 read)
                    bias_b = cst.tile([P, OFULL], f32)
                    nc.sync.dma_start(
                        out=bias_b[:], in_=bass.AP(bias, 0, [[0, P], [1, OFULL]])
                    )
                sc16, zr16 = band_extract(b, wq_t, sc, zr)
                if pend is not None:
                    band_scalefix(pend[0], pend[1], pend[2])
                pend = (b, sc16, zr16)
            band_scalefix(pend[0], pend[1], pend[2])

            # --- main GEMM loop ---
            for ns in range(NSUP):
                tok0 = ns * SUP
                xb = xbp.tile([P, KT, SUP], bf16, tag="xb")
                for kt in range(KT):
                    xf = xp.tile([P, SUP], f32, tag="xf")
                    nc.sync.dma_start(
                        out=xf[:], in_=xt[kt * P:(kt + 1) * P, tok0:tok0 + SUP]
                    )
                    if USE_GPS_XCAST:
                        nc.gpsimd.tensor_copy(out=xb[:, kt, :], in_=xf[:])
                    else:
                        nc.scalar.copy(out=xb[:, kt, :], in_=xf[:])
                for sub in range(NSUB):
                    ps = [
                        pp.tile([P, sz], f32, tag=f"ps{ci}", name=f"ps{ci}")
                        for ci, (off, sz) in enumerate(CHUNKS)
                    ]
                    for kt in range(KT):
                        lhsT = xb[:, kt, sub * P:(sub + 1) * P]
                        w_b = w_bands[kt // KTB]
                        j = kt % KTB
                        for ci, (off, sz) in enumerate(CHUNKS):
                            nc.tensor.matmul(
                                ps[ci][:],
                                lhsT,
                                w_b[:, j, off:off + sz],
                                start=(kt == 0),
                                stop=(kt == KT - 1),
                            )
                    ot = op.tile([P, OFULL], f32, tag="ot")
                    row0 = tok0 + sub * P
                    for ci, (off, sz) in enumerate(CHUNKS):
                        nc.vector.tensor_add(
                            out=ot[:, off:off + sz], in0=ps[ci][:],
                            in1=bias_b[:, off:off + sz],
                        )
                        nc.sync.dma_start(
                            out=out[row0:row0 + P, off:off + sz],
                            in_=ot[:, off:off + sz],
                        )

    nc.compile()
    return nc


_PROG_CACHE = {}


def _get_program():
    key = "full"
    if key not in _PROG_CACHE:
        _PROG_CACHE[key] = build_program()
    return _PROG_CACHE[key]


def shard_inputs(x, W_q, scale, zero, bias):
    """Host-side sharding / layout transforms (no arithmetic on values)."""
    x = np.asarray(x, dtype=np.float32)
    W_q = np.asarray(W_q)
    scale = np.asarray(scale, dtype=np.float32)
    zero = np.asarray(zero, dtype=np.float32)
    bias = np.asarray(bias, dtype=np.float32)

    ntok = x.shape[0] * x.shape[1]
    xt = np.ascontiguousarray(x.reshape(ntok, IN_F).T)              # [K, NTOK]
    scale_t = np.ascontiguousarray(scale.reshape(R_FULL, IN_F).T)   # [K, R]
    zero_t = np.ascontiguousarray(zero.reshape(R_FULL, IN_F).T)     # [K, R]
    wq_u8 = W_q.astype(np.uint8)                                    # values < 256
    bias_rs = bias.reshape(GROUP, R_FULL)                           # [i, r]

    in_maps = []
    for c in range(N_CORES):
        rows = wq_u8[IB_FULL * c: IB_FULL * (c + 1)]                # [4, 704512]
        # [ib, r, k] -> [k, ib, r] -> [K, OHALF]
        wq_c = np.ascontiguousarray(
            rows.reshape(IB_FULL, R_FULL, IN_F).transpose(2, 0, 1)
        ).reshape(IN_F, IB_FULL * R_FULL)
        bias_c = np.concatenate(
            [
                bias_rs[IB_FULL * c: IB_FULL * (c + 1)].ravel(),
                bias_rs[32 + IB_FULL * c: 32 + IB_FULL * (c + 1)].ravel(),
            ]
        )
        in_maps.append(
            {
                "xt": xt,
                "wq": wq_c,
                "scale_t": scale_t,
                "zero_t": zero_t,
                "bias": bias_c,
            }
        )
    return in_maps


def gather_output(results, ntok=NTOK_FULL):
    out = np.empty((ntok, OUT_F), dtype=np.float32)
    ohalf = IB_FULL * R_FULL
    for c in range(N_CORES):
        res = results[c]["out"]
        lo = IB_FULL * c * R_FULL
        out[:, lo: lo + ohalf] = res[:, :ohalf]
        lo = (32 + IB_FULL * c) * R_FULL
        out[:, lo: lo + ohalf] = res[:, ohalf:]
    return out


def kernel(x, W_q, scale, zero, bias):
    from concourse.bass_utils import run_bass_kernel_spmd

    x = np.asarray(x)
    b, s, _ = x.shape
    nc = _get_program()
    in_maps = shard_inputs(x, W_q, scale, zero, bias)
    res = run_bass_kernel_spmd(nc, in_maps, list(range(N_CORES)))
    out = gather_output(res.results)
    return out.reshape(b, s, OUT_F)
